# revision 45
# baseline (speedup 1.0000x reference)
"""Trainium2 Bass kernel for nn_MiniGRUConv2d4 (MinGRU 4-direction conv scan).

Problem (B=4, Cin=64, Cout4=256, H=W=256):
    u_c  = conv3x3(xs, w_c) + bn_c          for c in {z, h, s}   (Cout=256)
    z    = sigmoid(u_z); hh = u_h; s = sigmoid(u_s)
    split 256 channels into 4 groups of 64; group g scans
      g=0: over H fwd, g=1: over H rev, g=2: over W fwd, g=3: over W rev
      h_i = z_i*hh_i + (1-z_i)*h_{i-1}
    out  = sum_g s_g * h_g                  (B, 64, H, W)

Sharding (8 cores): core = (batch b, orientation o).
  o=0: natural image, conv channels 128..255 (groups 2,3: W-fwd / W-rev)
  o=1: transposed image (host transposes), channels 0..127 (groups 0,1:
       H-scan becomes W-scan in the transposed frame).

Device layout: each conv PSUM tile holds ONE scan group (64 chans) in
row-split form — partitions = (row-half h, chan c), filled by two
concurrent M=64 matmuls (col-group tiling via out.base_partition 0/64)
whose rhs streams come from different band rows. The scan then runs at
full 128-partition width directly on conv output (no SBUF remap). The
conv is 4 K=128 fp16 waves per (j, conv, group) tile (dy0/dy1 tap
pairs packed into the 128-partition contraction via a row-shifted
second input copy; dy2 dx0/dx1 partition-packed from x3) plus grouped
K=64 "leftover" pair-waves for dy2-dx2: two tiles' leftovers land on
disjoint row groups (x3 holds that tap in both halves: lower at col+2,
upper at col+1 from the host-preshifted xp2) and run concurrently as 4
disjoint 64x64 quadrants — 27 waves per j-step vs the naive 30. Edge
bands are half-height to shrink the startup transfer and the serial
tail chain; ~12 dummy matmuls on zeroed scratch warm the HAM clock
gate during the startup DMA wait. Host folds BN into weights/biases,
pads, transposes, builds the col-shifted xp2 copy, and sums the
per-core partial products. ~93% PE-busy, ~369 us PE-stream floor.
"""

import sys
import types

import numpy as np

import concourse.bass as bass
import concourse.mybir as mybir
import concourse.tile as tile

F32 = mybir.dt.float32
AF = mybir.ActivationFunctionType
OP = mybir.AluOpType

_R = 8  # band height (output rows per band)


# ---------------------------------------------------------------------------
# Workaround: the pinned walrus rejects instructions carrying more than a
# couple of sem waits ("Too many sync wait commands", CoreV3GenImpl
# setupSyncWait). Hoist excess waits onto same-engine NOPs inserted right
# before the offending instruction.
_MAX_WAITS = 1


def _split_excess_waits(nc, max_waits=_MAX_WAITS):
    import bass_rust

    n_split = 0
    for f in nc.m.functions:
        for blk in f.blocks:
            out = []
            for inst in blk.instructions:
                si = inst.sync_info
                if si is not None and len(si.on_wait) > max_waits:
                    waits = list(si.on_wait)
                    extra, keep = waits[:-max_waits], waits[-max_waits:]
                    for i0 in range(0, len(extra), max_waits):
                        nop = mybir.InstNoOp(
                            name=f"{inst.name}_xw{i0}", ins=[], outs=[]
                        )
                        nop.engine = inst.engine
                        nop.sync_info = bass_rust.SyncInfo(
                            on_wait=extra[i0 : i0 + max_waits], on_update=[]
                        )
                        nc.register_instruction(nop)
                        out.append(nop)
                        n_split += 1
                    inst.sync_info = bass_rust.SyncInfo(
                        on_wait=keep, on_update=list(si.on_update)
                    )
                out.append(inst)
            blk.instructions = out
    return n_split


def _ensure_axon_hooks_importable():
    # bass_utils imports antenv.axon_hooks when tracing is requested; the
    # container's antenv stub lacks it. Provide a no-op registry so the
    # import never crashes (tracing then just degrades gracefully).
    try:
        import antenv.axon_hooks  # noqa: F401
    except Exception:
        try:
            import antenv

            mod = types.ModuleType("antenv.axon_hooks")
            mod._hook = None
            mod.set_axon_ntff_profile_hook = lambda h: setattr(mod, "_hook", h)
            mod.get_axon_ntff_profile_hook = lambda: mod._hook
            sys.modules["antenv.axon_hooks"] = mod
            antenv.axon_hooks = mod
        except Exception:
            pass


# ---------------------------------------------------------------------------
# Device program

# Conv operands: fp16 runs the PE at full rate (1 cyc/row, like bf16) but
# carries a 10-bit mantissa — conv error ~5e-4 vs bf16's ~3e-3. fp32r would
# be exact-ish but its fused 4-byte weight load can't pipeline.
CONV_DT = mybir.dt.float16
CHAIN_DT = mybir.dt.float16  # z/s/a/b/h/p tiles + output (host upcasts)
WPOOL_BUFS = 4
XPOOL_BUFS = 4


def build_nc(H, W, with_init_fixup=True):
    """One-core program; all 8 cores run it SPMD with different inputs."""
    R = _R
    RR = R + 1  # input rows resident per band (dy0/dy1 buffer)
    Wp = W + 2
    assert H % R == 0 and W % 2 == 0
    # band list (y0, R_b): half-height bands at both edges halve the
    # startup-gating first transfer and the serial tail chain
    Rh2 = R // 2
    bands = [(0, Rh2), (Rh2, Rh2)]
    y = R
    while y < H - R:
        bands.append((y, R))
        y += R
    bands.append((y, Rh2))
    bands.append((y + Rh2, Rh2))
    nbands = len(bands)
    FWmax = (R // 2) * W
    cdt = CONV_DT
    wdt = CHAIN_DT

    nc = bass.Bass("TRN2", target_bir_lowering=False, debug=False)
    xp = nc.dram_tensor("xp", [64, H + 2, Wp], cdt, kind="ExternalInput").ap()
    # col+1-shifted copy (host-built): lets the x3-upper load read full
    # contiguous rows -> 8x bigger DMA packets than a shifted slice of xp
    xp2 = nc.dram_tensor("xp2", [64, H + 2, Wp], cdt, kind="ExternalInput").ap()
    # weights split per scan group so the first tiles' (group 0) half
    # arrives in ~1.2us and gates the first matmul as little as possible
    wts_a = nc.dram_tensor("wts_a", [128, 15, 64], cdt, kind="ExternalInput").ap()
    wts_b = nc.dram_tensor("wts_b", [128, 15, 64], cdt, kind="ExternalInput").ap()
    consts = nc.dram_tensor("consts", [128, 8], F32, kind="ExternalInput").ap()
    # out free dim: [band][group][half-rows x W]; partitions = (half, chan)
    out = nc.dram_tensor("out", [128, 2 * H * W // 128 * 64], wdt,
                         kind="ExternalOutput").ap()
    # 2*H*W*64/128 = H*W: per-partition free size is nbands * 2 * FW = H*W

    with tile.TileContext(nc) as tc:
        with (
            tc.tile_pool(name="const", bufs=1) as cpool,
            tc.tile_pool(name="xin", bufs=XPOOL_BUFS) as xpool,
            tc.tile_pool(name="work", bufs=WPOOL_BUFS) as wpool,
            tc.tile_pool(name="psum", bufs=2, space="PSUM") as ppool,
        ):
            # weights/consts ride the Scalar hwdge queue so their transfer
            # overlaps band 0's x loads (issued on Sync below); group-0
            # weights first — they gate the first matmul
            wsb = [cpool.tile([128, 15, 64], cdt, name=f"wsb{g}")
                   for g in range(2)]
            nc.scalar.dma_start(wsb[0][:], wts_a)
            cst = cpool.tile([128, 8], F32)
            nc.scalar.dma_start(cst[:], consts)
            nc.scalar.dma_start(wsb[1][:], wts_b)

            # preload the sigmoid table (lazy ACT_TABLE_LOAD costs 1.3us
            # on the first real ACTIVATE otherwise)
            scr1 = cpool.tile([128, 1], cdt)
            nc.vector.memset(scr1[:], 0.0)
            nc.scalar.activation(scr1[:], scr1[:], AF.Sigmoid)
            # per-group bias vectors in (half, chan) layout
            bias = [[cst[:, 3 * g + c : 3 * g + c + 1] for c in range(3)]
                    for g in range(2)]  # bias[g][conv]
            init = [cst[:, 6 + g : 7 + g] for g in range(2)]

            def load_x(b):
                y0, Rb = bands[b]
                # x2: dy0 rows at partitions 0:64, dy1 rows at 64:128
                # (tiles are max-band-sized; short bands use a row subset)
                x2 = xpool.tile([128, RR, Wp], cdt, name="x2")
                nc.sync.dma_start(x2[0:64, 0 : Rb + 1], xp[:, y0 : y0 + Rb + 1, :])
                nc.sync.dma_start(
                    x2[64:128, 0 : Rb + 1], xp[:, y0 + 1 : y0 + 2 + Rb, :]
                )
                # x3: dy2 rows; lower = col+0, upper = col+1 (from the
                # pre-shifted xp2 so the transfer is row-contiguous)
                x3 = xpool.tile([128, R, Wp], cdt, name="x3")
                nc.sync.dma_start(x3[0:64, 0:Rb], xp[:, y0 + 2 : y0 + 2 + Rb, :])
                nc.sync.dma_start(x3[64:128, 0:Rb], xp2[:, y0 + 2 : y0 + 2 + Rb, :])
                return x2, x3

            # x loads run 3 bands ahead of use, emitted BEFORE the previous
            # bands' out-stores so the sync queue never head-of-line blocks
            # a prefetch behind a store that waits on the scan chain.
            xq = {b: load_x(b) for b in range(min(2, nbands))}
            for band in range(nbands):
                if band + 2 < nbands:
                    xq[band + 2] = load_x(band + 2)
                x2, x3 = xq.pop(band)
                y0, Rb = bands[band]
                Rh = Rb // 2  # rows per half-band
                FW = Rh * W  # free width of one half-band slab

                # per-group work tiles, all in (half, chan) partition layout;
                # free dim = Rh rows x W cols, row-major
                z_t = [wpool.tile([128, FWmax], wdt, name=f"z{g}") for g in range(2)]
                s_t = [wpool.tile([128, FWmax], wdt, name=f"s{g}") for g in range(2)]
                # [a | b] contiguous so the scan reads one tile
                ab_t = [wpool.tile([128, 2 * FWmax], wdt, name=f"ab{g}") for g in range(2)]
                h_t = [wpool.tile([128, FWmax], wdt, name=f"h{g}") for g in range(2)]
                # p for both groups in one tile
                p_t = wpool.tile([128, 2 * FWmax], wdt)

                def consume(g, us, sl, defer_s=False):
                    # PSUM -> SBUF consumers for one (j, group). On the
                    # last band the s-sigmoid (only needed for the final
                    # multiply) is deferred so the a/edge/scan chain gets
                    # the ACT engine first.
                    if not defer_s:
                        nc.scalar.activation(
                            s_t[g][:, sl], us[g][2][:], AF.Sigmoid,
                            bias=bias[g][2],
                        )
                    nc.scalar.activation(
                        z_t[g][:, sl], us[g][0][:], AF.Sigmoid, bias=bias[g][0]
                    )
                    # b = (u_h + bias_h) * z
                    nc.vector.scalar_tensor_tensor(
                        ab_t[g][:, FW + sl.start : FW + sl.stop],
                        us[g][1][:], bias[g][1], z_t[g][:, sl],
                        op0=OP.add, op1=OP.mult,
                    )

                # on the last band, defer group 1's final consumers until
                # after group 0's scan chain is queued, so the tail
                # serial chain (ACT a/edge -> scan) starts ~3us earlier
                tail_split = band == nbands - 1

                for j in range(Rh // 2):  # j covers rows (2j, 2j+1) per half
                    sl = slice(j * 2 * W, (j + 1) * 2 * W)
                    # K=128-dense waves for all 6 (group, conv) PSUM tiles,
                    # with the dy2-dx2 leftovers emitted as paired K=64
                    # matmuls after every SECOND tile: x3 holds that tap in
                    # BOTH partition halves (lower at col+2, upper at
                    # col+1), so two tiles' leftovers land on disjoint row
                    # groups and run concurrently (4 disjoint 64x64
                    # quadrants per wave). Pairing mid-sequence (not at the
                    # end of the j-step) lets each PSUM tile stop early so
                    # its ACT/DVE consumer frees it before the next j-step
                    # needs the bank.
                    us = [[None] * 3 for _ in range(2)]
                    # Tile order chosen so the NEXT j-step's bank reuse
                    # never stalls: it starts with the double-buffered
                    # h-conv tiles, and the single-buffered tiles' consumers
                    # (emitted in matching order below) finish before their
                    # banks are needed again.
                    tiles = [(0, 1), (0, 2), (0, 0), (1, 1), (1, 2), (1, 0)]
                    for g, c in tiles:
                        u = ppool.tile(
                            [128, 2 * W], F32, name=f"u{c}g{g}",
                            tag=f"u{c}g{g}", bufs=(2 if c == 1 else 1),
                        )
                        us[g][c] = u
                        for hh in range(2):  # concurrent col-group halves
                            r0 = hh * Rh + 2 * j
                            p0, p1 = 64 * hh, 64 * hh + 64
                            for dx in range(3):  # dy0+dy1 pairs
                                nc.tensor.matmul(
                                    u[p0:p1],
                                    wsb[g][:, 3 * c + dx, :],
                                    x2[:, r0 : r0 + 2, dx : dx + W],
                                    start=(dx == 0),
                                    stop=False,
                                )
                            # dy2 (dx0, dx1) pair
                            nc.tensor.matmul(
                                u[p0:p1],
                                wsb[g][:, 9 + c, :],
                                x3[:, r0 : r0 + 2, 0:W],
                                start=False,
                                stop=False,
                            )
                    # dy2-dx2 leftovers as one grouped block of 3 pair-waves
                    # (K=64, 4 disjoint quadrants each): grouping pays the
                    # row-tiled LDWEIGHTS drain-stall once instead of per
                    # pair.
                    for i in range(0, 6, 2):
                        for k, (gg, cc) in enumerate(tiles[i : i + 2]):
                            k0 = 64 * k  # first: low half, second: up
                            xoff = 2 - k  # low: col+2, up: col+1
                            for hh in range(2):
                                r0 = hh * Rh + 2 * j
                                p0, p1 = 64 * hh, 64 * hh + 64
                                nc.tensor.matmul(
                                    us[gg][cc][p0:p1],
                                    wsb[gg][k0 : k0 + 64, 12 + cc, :],
                                    x3[k0 : k0 + 64, r0 : r0 + 2,
                                       xoff : xoff + W],
                                    start=False,
                                    stop=True,
                                )
                    # consumers in PSUM-availability order (s-g0's bank is
                    # the first one the next j-step reuses)
                    for g in range(2):
                        consume(
                            g, us, sl,
                            defer_s=(tail_split and j == Rh // 2 - 1),
                        )
                    last_us, last_sl = us, sl

                for g in range(2):
                    a_f = ab_t[g][:, 0:FW]
                    b_f = ab_t[g][:, FW : 2 * FW]
                    # a = 1 - z (on ACT: Identity(-z + 1))
                    nc.scalar.activation(
                        a_f, z_t[g][:, 0:FW], AF.Identity, bias=1.0, scale=-1.0
                    )
                    a3 = a_f.rearrange("p (r w) -> p r w", w=W)
                    b3 = b_f.rearrange("p (r w) -> p r w", w=W)
                    edge = 0 if g == 0 else W - 1
                    # fold the (normally zero) scan init into b at each row
                    # edge, then zero `a` there so the flat scan restarts
                    # per row.
                    if with_init_fixup:
                        nc.vector.scalar_tensor_tensor(
                            b3[:, :, edge], a3[:, :, edge], init[g],
                            b3[:, :, edge], op0=OP.mult, op1=OP.add,
                        )
                    nc.scalar.activation(
                        a3[:, :, edge], a3[:, :, edge], AF.Copy,
                        bias=0.0, scale=0.0,
                    )
                    # scan at full 128-partition width; group 1 scans
                    # backward via reversed APs
                    if g == 0:
                        nc.vector.tensor_tensor_scan(
                            h_t[g][:, 0:FW], a_f, b_f, 0.0,
                            op0=OP.mult, op1=OP.add,
                        )
                    else:
                        nc.vector.tensor_tensor_scan(
                            h_t[g][:, 0:FW][:, ::-1], a_f[:, ::-1],
                            b_f[:, ::-1], 0.0, op0=OP.mult, op1=OP.add,
                        )
                    if tail_split:
                        # deferred s-sigmoid (see consume)
                        nc.scalar.activation(
                            s_t[g][:, last_sl], last_us[g][2][:], AF.Sigmoid,
                            bias=bias[g][2],
                        )
                    nc.vector.tensor_mul(
                        p_t[:, g * FW : (g + 1) * FW], s_t[g][:, 0:FW],
                        h_t[g][:, 0:FW],
                    )
                    # out store per group on the sync hardware queue (the
                    # gpsimd software DGE dribbles straggler packets ~10us
                    # late); prefetches are emitted ahead of stores above.
                    nc.sync.dma_start(
                        out[:, y0 * W + g * FW : y0 * W + (g + 1) * FW],
                        p_t[:, g * FW : (g + 1) * FW],
                    )
    _split_excess_waits(nc)
    return nc


# ---------------------------------------------------------------------------
# Host side

_NC_CACHE = {}


def _get_nc(H, W, with_init_fixup=True):
    key = (H, W, with_init_fixup)
    if key not in _NC_CACHE:
        _NC_CACHE[key] = build_nc(H, W, with_init_fixup)
    return _NC_CACHE[key]


def make_in_maps(inputs, H, W):
    """Build the 8 per-core input dicts from the full problem inputs."""
    xs = np.ascontiguousarray(np.asarray(inputs["xs"], dtype=np.float32))
    B = xs.shape[0]
    Ws, Bs = {}, {}
    for tag in ("z", "h", "s"):
        w = np.asarray(inputs["w_" + tag], dtype=np.float32)
        g = np.asarray(inputs["g_" + tag], dtype=np.float32)
        be = np.asarray(inputs["b_" + tag], dtype=np.float32)
        m = np.asarray(inputs["m_" + tag], dtype=np.float32)
        v = np.asarray(inputs["v_" + tag], dtype=np.float32)
        inv = g / np.sqrt(v + 1e-5)
        Ws[tag] = w * inv[:, None, None, None]
        Bs[tag] = be - m * inv
    init = {
        k: np.asarray(inputs[k], dtype=np.float32).reshape(-1)
        for k in ("h20", "h21", "h30", "h31")
    }

    in_maps = []
    for b in range(B):
        for orient in (0, 1):
            if orient == 0:
                img = xs[b]
                ch = slice(128, 256)
                init_a, init_b = init["h30"], init["h31"]
            else:
                img = xs[b].transpose(0, 2, 1)
                ch = slice(0, 128)
                init_a, init_b = init["h20"], init["h21"]
            xpad = np.pad(img, ((0, 0), (1, 1), (1, 1)))
            wts = np.zeros((128, 15, 128), np.float32)
            consts = np.zeros((128, 8), np.float32)
            for c, tag in enumerate(("z", "h", "s")):
                wc = Ws[tag][ch]  # (128, 64, 3, 3) [cout, cin, ky, kx]
                if orient == 1:
                    wc = wc.transpose(0, 1, 3, 2)
                for dx in range(3):
                    wts[0:64, 3 * c + dx, :] = wc[:, :, 0, dx].T
                    wts[64:128, 3 * c + dx, :] = wc[:, :, 1, dx].T
                wts[0:64, 9 + c, :] = wc[:, :, 2, 0].T
                wts[64:128, 9 + c, :] = wc[:, :, 2, 1].T
                # dy2 dx2 weights in BOTH halves: the K=64 leftover matmuls
                # read it from either row half (paired for concurrency)
                wts[0:64, 12 + c, :] = wc[:, :, 2, 2].T
                wts[64:128, 12 + c, :] = wc[:, :, 2, 2].T
                # biases in (half, chan) layout, separate per scan group
                bg = Bs[tag][ch]
                consts[0:64, c] = bg[0:64]
                consts[64:128, c] = bg[0:64]
                consts[0:64, 3 + c] = bg[64:128]
                consts[64:128, 3 + c] = bg[64:128]
            consts[0:64, 6] = init_a
            consts[64:128, 6] = init_a
            consts[0:64, 7] = init_b
            consts[64:128, 7] = init_b
            cnp = mybir.dt.np(CONV_DT)
            if xpad.dtype != cnp:
                xpad = xpad.astype(cnp)
                wts = wts.astype(cnp)
            # col+1-shifted copy for the row-contiguous x3-upper load
            xpad2 = np.empty_like(xpad)
            xpad2[:, :, :-1] = xpad[:, :, 1:]
            xpad2[:, :, -1] = 0
            in_maps.append(
                {
                    "xp": np.ascontiguousarray(xpad),
                    "xp2": np.ascontiguousarray(xpad2),
                    "wts_a": np.ascontiguousarray(wts[:, :, 0:64]),
                    "wts_b": np.ascontiguousarray(wts[:, :, 64:128]),
                    "consts": consts,
                }
            )
    return in_maps


def _band_list(H):
    R, Rh2 = _R, _R // 2
    bands = [(0, Rh2), (Rh2, Rh2)]
    y = R
    while y < H - R:
        bands.append((y, R))
        y += R
    bands.append((y, Rh2))
    bands.append((y + Rh2, Rh2))
    return bands


def gather_output(core_outs, B, H, W):
    """core_outs: list of 8 arrays (128, H*W) in core order (b-major).

    Device layout: partitions = (half hh in {0,1}, chan c in 0..63);
    free = [band][group g in {0,1}][row r in 0..Rh_b-1][col]. Global row
    of an element = y0_b + hh*Rh_b + r.
    """
    bands = _band_list(H)
    out = np.empty((B, 64, H, W), np.float32)
    for b in range(B):
        for orient in (0, 1):
            flat = core_outs[2 * b + orient].astype(np.float32)
            o = np.empty((64, H, W), np.float32)
            for y0, Rb in bands:
                Rh = Rb // 2
                seg = flat[:, y0 * W : (y0 + Rb) * W]
                seg = seg.reshape(2, 64, 2, Rh, W)  # hh, c, g, r, w
                seg = seg.sum(axis=2)  # sum scan groups: hh, c, r, w
                o[:, y0 : y0 + Rb] = seg.transpose(1, 0, 2, 3).reshape(
                    64, Rb, W
                )
            if orient == 0:
                out[b] = o
            else:
                out[b] += o.transpose(0, 2, 1)
    return out


def kernel(**inputs):
    from concourse.bass_utils import run_bass_kernel_spmd

    _ensure_axon_hooks_importable()
    xs = inputs["xs"]
    B, C, H, W = xs.shape
    # the scan-init fixup ops are only needed for nonzero initial states
    # (the problem spec ships all-zero inits)
    need_fixup = any(
        np.any(np.asarray(inputs[k], dtype=np.float32))
        for k in ("h20", "h21", "h30", "h31")
    )
    nc = _get_nc(H, W, with_init_fixup=need_fixup)
    in_maps = make_in_maps(inputs, H, W)
    res = run_bass_kernel_spmd(nc, in_maps, core_ids=list(range(len(in_maps))))
    outs = [res.results[c]["out"] for c in range(len(in_maps))]
    return gather_output(outs, B, H, W)


# revision 46
# speedup vs baseline: 1.0070x; 1.0070x over previous
"""Trainium2 Bass kernel for nn_MiniGRUConv2d4 (MinGRU 4-direction conv scan).

Problem (B=4, Cin=64, Cout4=256, H=W=256):
    u_c  = conv3x3(xs, w_c) + bn_c          for c in {z, h, s}   (Cout=256)
    z    = sigmoid(u_z); hh = u_h; s = sigmoid(u_s)
    split 256 channels into 4 groups of 64; group g scans
      g=0: over H fwd, g=1: over H rev, g=2: over W fwd, g=3: over W rev
      h_i = z_i*hh_i + (1-z_i)*h_{i-1}
    out  = sum_g s_g * h_g                  (B, 64, H, W)

Sharding (8 cores): core = (batch b, orientation o).
  o=0: natural image, conv channels 128..255 (groups 2,3: W-fwd / W-rev)
  o=1: transposed image (host transposes), channels 0..127 (groups 0,1:
       H-scan becomes W-scan in the transposed frame).

Device layout: each conv PSUM tile holds ONE scan group (64 chans) in
row-split form — partitions = (row-half h, chan c), filled by two
concurrent M=64 matmuls (col-group tiling via out.base_partition 0/64)
whose rhs streams come from different band rows. The scan then runs at
full 128-partition width directly on conv output (no SBUF remap). The
conv is 4 K=128 fp16 waves per (j, conv, group) tile (dy0/dy1 tap
pairs packed into the 128-partition contraction via a row-shifted
second input copy; dy2 dx0/dx1 partition-packed from x3) plus grouped
K=64 "leftover" pair-waves for dy2-dx2: two tiles' leftovers land on
disjoint row groups (x3 holds that tap in both halves: lower at col+2,
upper at col+1 from the host-preshifted xp2) and run concurrently as 4
disjoint 64x64 quadrants — 27 waves per j-step vs the naive 30. Edge
bands are half-height to shrink the startup transfer and the serial
tail chain; ~12 dummy matmuls on zeroed scratch warm the HAM clock
gate during the startup DMA wait. Host folds BN into weights/biases,
pads, transposes, builds the col-shifted xp2 copy, and sums the
per-core partial products. ~93% PE-busy, ~369 us PE-stream floor.
"""

import sys
import types

import numpy as np

import concourse.bass as bass
import concourse.mybir as mybir
import concourse.tile as tile

F32 = mybir.dt.float32
AF = mybir.ActivationFunctionType
OP = mybir.AluOpType

_R = 8  # band height (output rows per band)


# ---------------------------------------------------------------------------
# Workaround: the pinned walrus rejects instructions carrying more than a
# couple of sem waits ("Too many sync wait commands", CoreV3GenImpl
# setupSyncWait). Hoist excess waits onto same-engine NOPs inserted right
# before the offending instruction.
_MAX_WAITS = 1


def _split_excess_waits(nc, max_waits=_MAX_WAITS):
    import bass_rust

    n_split = 0
    for f in nc.m.functions:
        for blk in f.blocks:
            out = []
            for inst in blk.instructions:
                si = inst.sync_info
                if si is not None and len(si.on_wait) > max_waits:
                    waits = list(si.on_wait)
                    extra, keep = waits[:-max_waits], waits[-max_waits:]
                    for i0 in range(0, len(extra), max_waits):
                        nop = mybir.InstNoOp(
                            name=f"{inst.name}_xw{i0}", ins=[], outs=[]
                        )
                        nop.engine = inst.engine
                        nop.sync_info = bass_rust.SyncInfo(
                            on_wait=extra[i0 : i0 + max_waits], on_update=[]
                        )
                        nc.register_instruction(nop)
                        out.append(nop)
                        n_split += 1
                    inst.sync_info = bass_rust.SyncInfo(
                        on_wait=keep, on_update=list(si.on_update)
                    )
                out.append(inst)
            blk.instructions = out
    return n_split


def _ensure_axon_hooks_importable():
    # bass_utils imports antenv.axon_hooks when tracing is requested; the
    # container's antenv stub lacks it. Provide a no-op registry so the
    # import never crashes (tracing then just degrades gracefully).
    try:
        import antenv.axon_hooks  # noqa: F401
    except Exception:
        try:
            import antenv

            mod = types.ModuleType("antenv.axon_hooks")
            mod._hook = None
            mod.set_axon_ntff_profile_hook = lambda h: setattr(mod, "_hook", h)
            mod.get_axon_ntff_profile_hook = lambda: mod._hook
            sys.modules["antenv.axon_hooks"] = mod
            antenv.axon_hooks = mod
        except Exception:
            pass


# ---------------------------------------------------------------------------
# Device program

# Conv operands: fp16 runs the PE at full rate (1 cyc/row, like bf16) but
# carries a 10-bit mantissa — conv error ~5e-4 vs bf16's ~3e-3. fp32r would
# be exact-ish but its fused 4-byte weight load can't pipeline.
CONV_DT = mybir.dt.float16
CHAIN_DT = mybir.dt.float16  # z/s/a/b/h/p tiles + output (host upcasts)
WPOOL_BUFS = 4
XPOOL_BUFS = 4


def build_nc(H, W, with_init_fixup=True):
    """One-core program; all 8 cores run it SPMD with different inputs."""
    R = _R
    RR = R + 1  # input rows resident per band (dy0/dy1 buffer)
    Wp = W + 2
    assert H % R == 0 and W % 2 == 0
    # band list (y0, R_b): half-height bands at both edges halve the
    # startup-gating first transfer and the serial tail chain
    Rh2 = R // 2
    bands = [(0, Rh2), (Rh2, Rh2)]
    y = R
    while y < H - R:
        bands.append((y, R))
        y += R
    bands.append((y, Rh2))
    bands.append((y + Rh2, Rh2))
    nbands = len(bands)
    FWmax = (R // 2) * W
    cdt = CONV_DT
    wdt = CHAIN_DT

    nc = bass.Bass("TRN2", target_bir_lowering=False, debug=False)
    xp = nc.dram_tensor("xp", [64, H + 2, Wp], cdt, kind="ExternalInput").ap()
    # col+1-shifted copy (host-built): lets the x3-upper load read full
    # contiguous rows -> 8x bigger DMA packets than a shifted slice of xp
    xp2 = nc.dram_tensor("xp2", [64, H + 2, Wp], cdt, kind="ExternalInput").ap()
    # weights split per scan group so the first tiles' (group 0) half
    # arrives in ~1.2us and gates the first matmul as little as possible
    wts_a = nc.dram_tensor("wts_a", [128, 15, 64], cdt, kind="ExternalInput").ap()
    wts_b = nc.dram_tensor("wts_b", [128, 15, 64], cdt, kind="ExternalInput").ap()
    consts = nc.dram_tensor("consts", [128, 8], F32, kind="ExternalInput").ap()
    # out free dim: [band][group][half-rows x W]; partitions = (half, chan)
    out = nc.dram_tensor("out", [128, 2 * H * W // 128 * 64], wdt,
                         kind="ExternalOutput").ap()
    # 2*H*W*64/128 = H*W: per-partition free size is nbands * 2 * FW = H*W

    with tile.TileContext(nc) as tc:
        with (
            tc.tile_pool(name="const", bufs=1) as cpool,
            tc.tile_pool(name="xin", bufs=XPOOL_BUFS) as xpool,
            tc.tile_pool(name="work", bufs=WPOOL_BUFS) as wpool,
            tc.tile_pool(name="psum", bufs=2, space="PSUM") as ppool,
        ):
            # weights/consts ride the Scalar hwdge queue so their transfer
            # overlaps band 0's x loads (issued on Sync below); group-0
            # weights first — they gate the first matmul
            wsb = [cpool.tile([128, 15, 64], cdt, name=f"wsb{g}")
                   for g in range(2)]
            nc.scalar.dma_start(wsb[0][:], wts_a)
            cst = cpool.tile([128, 8], F32)
            nc.scalar.dma_start(cst[:], consts)
            nc.scalar.dma_start(wsb[1][:], wts_b)

            # preload the sigmoid table (lazy ACT_TABLE_LOAD costs 1.3us
            # on the first real ACTIVATE otherwise)
            scr1 = cpool.tile([128, 1], cdt)
            nc.vector.memset(scr1[:], 0.0)
            nc.scalar.activation(scr1[:], scr1[:], AF.Sigmoid)

            # PE warm-up: the HAM clock gate holds the PE at 1.2 GHz until
            # ~3.4us of sustained matmul activity, and band-0 data only
            # lands ~12.5us in — burn the DMA wait on dummy matmuls over
            # zeroed scratch so the real stream starts at 2.4 GHz. (Real
            # accumulation groups open with start=True, clearing the bank.)
            scr = cpool.tile([128, 512], cdt)
            nc.vector.memset(scr[:], 0.0)
            for wi in range(12):
                warm = ppool.tile(
                    [128, 2 * W], F32, name="u1g0", tag="u1g0", bufs=2
                )
                nc.tensor.matmul(
                    warm[:], scr[:, 0:128], scr[:, 0:512],
                    start=True, stop=True,
                )
            # per-group bias vectors in (half, chan) layout
            bias = [[cst[:, 3 * g + c : 3 * g + c + 1] for c in range(3)]
                    for g in range(2)]  # bias[g][conv]
            init = [cst[:, 6 + g : 7 + g] for g in range(2)]

            def load_x(b):
                y0, Rb = bands[b]
                # x2: dy0 rows at partitions 0:64, dy1 rows at 64:128
                # (tiles are max-band-sized; short bands use a row subset)
                x2 = xpool.tile([128, RR, Wp], cdt, name="x2")
                nc.sync.dma_start(x2[0:64, 0 : Rb + 1], xp[:, y0 : y0 + Rb + 1, :])
                nc.sync.dma_start(
                    x2[64:128, 0 : Rb + 1], xp[:, y0 + 1 : y0 + 2 + Rb, :]
                )
                # x3: dy2 rows; lower = col+0, upper = col+1 (from the
                # pre-shifted xp2 so the transfer is row-contiguous)
                x3 = xpool.tile([128, R, Wp], cdt, name="x3")
                nc.sync.dma_start(x3[0:64, 0:Rb], xp[:, y0 + 2 : y0 + 2 + Rb, :])
                nc.sync.dma_start(x3[64:128, 0:Rb], xp2[:, y0 + 2 : y0 + 2 + Rb, :])
                return x2, x3

            # x loads run 3 bands ahead of use, emitted BEFORE the previous
            # bands' out-stores so the sync queue never head-of-line blocks
            # a prefetch behind a store that waits on the scan chain.
            xq = {b: load_x(b) for b in range(min(2, nbands))}
            for band in range(nbands):
                if band + 2 < nbands:
                    xq[band + 2] = load_x(band + 2)
                x2, x3 = xq.pop(band)
                y0, Rb = bands[band]
                Rh = Rb // 2  # rows per half-band
                FW = Rh * W  # free width of one half-band slab

                # per-group work tiles, all in (half, chan) partition layout;
                # free dim = Rh rows x W cols, row-major
                z_t = [wpool.tile([128, FWmax], wdt, name=f"z{g}") for g in range(2)]
                s_t = [wpool.tile([128, FWmax], wdt, name=f"s{g}") for g in range(2)]
                # [a | b] contiguous so the scan reads one tile
                ab_t = [wpool.tile([128, 2 * FWmax], wdt, name=f"ab{g}") for g in range(2)]
                h_t = [wpool.tile([128, FWmax], wdt, name=f"h{g}") for g in range(2)]
                # p for both groups in one tile
                p_t = wpool.tile([128, 2 * FWmax], wdt)

                def consume(g, us, sl, defer_s=False):
                    # PSUM -> SBUF consumers for one (j, group). On the
                    # last band the s-sigmoid (only needed for the final
                    # multiply) is deferred so the a/edge/scan chain gets
                    # the ACT engine first.
                    if not defer_s:
                        nc.scalar.activation(
                            s_t[g][:, sl], us[g][2][:], AF.Sigmoid,
                            bias=bias[g][2],
                        )
                    nc.scalar.activation(
                        z_t[g][:, sl], us[g][0][:], AF.Sigmoid, bias=bias[g][0]
                    )
                    # b = (u_h + bias_h) * z
                    nc.vector.scalar_tensor_tensor(
                        ab_t[g][:, FW + sl.start : FW + sl.stop],
                        us[g][1][:], bias[g][1], z_t[g][:, sl],
                        op0=OP.add, op1=OP.mult,
                    )

                # on the last band, defer group 1's final consumers until
                # after group 0's scan chain is queued, so the tail
                # serial chain (ACT a/edge -> scan) starts ~3us earlier
                tail_split = band == nbands - 1

                for j in range(Rh // 2):  # j covers rows (2j, 2j+1) per half
                    sl = slice(j * 2 * W, (j + 1) * 2 * W)
                    # K=128-dense waves for all 6 (group, conv) PSUM tiles,
                    # with the dy2-dx2 leftovers emitted as paired K=64
                    # matmuls after every SECOND tile: x3 holds that tap in
                    # BOTH partition halves (lower at col+2, upper at
                    # col+1), so two tiles' leftovers land on disjoint row
                    # groups and run concurrently (4 disjoint 64x64
                    # quadrants per wave). Pairing mid-sequence (not at the
                    # end of the j-step) lets each PSUM tile stop early so
                    # its ACT/DVE consumer frees it before the next j-step
                    # needs the bank.
                    us = [[None] * 3 for _ in range(2)]
                    # Tile order chosen so the NEXT j-step's bank reuse
                    # never stalls: it starts with the double-buffered
                    # h-conv tiles, and the single-buffered tiles' consumers
                    # (emitted in matching order below) finish before their
                    # banks are needed again.
                    tiles = [(0, 1), (0, 2), (0, 0), (1, 1), (1, 2), (1, 0)]
                    for g, c in tiles:
                        u = ppool.tile(
                            [128, 2 * W], F32, name=f"u{c}g{g}",
                            tag=f"u{c}g{g}", bufs=(2 if c == 1 else 1),
                        )
                        us[g][c] = u
                        for hh in range(2):  # concurrent col-group halves
                            r0 = hh * Rh + 2 * j
                            p0, p1 = 64 * hh, 64 * hh + 64
                            for dx in range(3):  # dy0+dy1 pairs
                                nc.tensor.matmul(
                                    u[p0:p1],
                                    wsb[g][:, 3 * c + dx, :],
                                    x2[:, r0 : r0 + 2, dx : dx + W],
                                    start=(dx == 0),
                                    stop=False,
                                )
                            # dy2 (dx0, dx1) pair
                            nc.tensor.matmul(
                                u[p0:p1],
                                wsb[g][:, 9 + c, :],
                                x3[:, r0 : r0 + 2, 0:W],
                                start=False,
                                stop=False,
                            )
                    # dy2-dx2 leftovers as one grouped block of 3 pair-waves
                    # (K=64, 4 disjoint quadrants each): grouping pays the
                    # row-tiled LDWEIGHTS drain-stall once instead of per
                    # pair.
                    for i in range(0, 6, 2):
                        for k, (gg, cc) in enumerate(tiles[i : i + 2]):
                            k0 = 64 * k  # first: low half, second: up
                            xoff = 2 - k  # low: col+2, up: col+1
                            for hh in range(2):
                                r0 = hh * Rh + 2 * j
                                p0, p1 = 64 * hh, 64 * hh + 64
                                nc.tensor.matmul(
                                    us[gg][cc][p0:p1],
                                    wsb[gg][k0 : k0 + 64, 12 + cc, :],
                                    x3[k0 : k0 + 64, r0 : r0 + 2,
                                       xoff : xoff + W],
                                    start=False,
                                    stop=True,
                                )
                    # consumers in PSUM-availability order (s-g0's bank is
                    # the first one the next j-step reuses)
                    for g in range(2):
                        consume(
                            g, us, sl,
                            defer_s=(tail_split and j == Rh // 2 - 1),
                        )
                    last_us, last_sl = us, sl

                for g in range(2):
                    a_f = ab_t[g][:, 0:FW]
                    b_f = ab_t[g][:, FW : 2 * FW]
                    # a = 1 - z (on ACT: Identity(-z + 1))
                    nc.scalar.activation(
                        a_f, z_t[g][:, 0:FW], AF.Identity, bias=1.0, scale=-1.0
                    )
                    a3 = a_f.rearrange("p (r w) -> p r w", w=W)
                    b3 = b_f.rearrange("p (r w) -> p r w", w=W)
                    edge = 0 if g == 0 else W - 1
                    # fold the (normally zero) scan init into b at each row
                    # edge, then zero `a` there so the flat scan restarts
                    # per row.
                    if with_init_fixup:
                        nc.vector.scalar_tensor_tensor(
                            b3[:, :, edge], a3[:, :, edge], init[g],
                            b3[:, :, edge], op0=OP.mult, op1=OP.add,
                        )
                    nc.scalar.activation(
                        a3[:, :, edge], a3[:, :, edge], AF.Copy,
                        bias=0.0, scale=0.0,
                    )
                    # scan at full 128-partition width; group 1 scans
                    # backward via reversed APs
                    if g == 0:
                        nc.vector.tensor_tensor_scan(
                            h_t[g][:, 0:FW], a_f, b_f, 0.0,
                            op0=OP.mult, op1=OP.add,
                        )
                    else:
                        nc.vector.tensor_tensor_scan(
                            h_t[g][:, 0:FW][:, ::-1], a_f[:, ::-1],
                            b_f[:, ::-1], 0.0, op0=OP.mult, op1=OP.add,
                        )
                    if tail_split:
                        # deferred s-sigmoid (see consume)
                        nc.scalar.activation(
                            s_t[g][:, last_sl], last_us[g][2][:], AF.Sigmoid,
                            bias=bias[g][2],
                        )
                    nc.vector.tensor_mul(
                        p_t[:, g * FW : (g + 1) * FW], s_t[g][:, 0:FW],
                        h_t[g][:, 0:FW],
                    )
                    # out store per group on the sync hardware queue (the
                    # gpsimd software DGE dribbles straggler packets ~10us
                    # late); prefetches are emitted ahead of stores above.
                    nc.sync.dma_start(
                        out[:, y0 * W + g * FW : y0 * W + (g + 1) * FW],
                        p_t[:, g * FW : (g + 1) * FW],
                    )
    _split_excess_waits(nc)
    return nc


# ---------------------------------------------------------------------------
# Host side

_NC_CACHE = {}


def _get_nc(H, W, with_init_fixup=True):
    key = (H, W, with_init_fixup)
    if key not in _NC_CACHE:
        _NC_CACHE[key] = build_nc(H, W, with_init_fixup)
    return _NC_CACHE[key]


def make_in_maps(inputs, H, W):
    """Build the 8 per-core input dicts from the full problem inputs."""
    xs = np.ascontiguousarray(np.asarray(inputs["xs"], dtype=np.float32))
    B = xs.shape[0]
    Ws, Bs = {}, {}
    for tag in ("z", "h", "s"):
        w = np.asarray(inputs["w_" + tag], dtype=np.float32)
        g = np.asarray(inputs["g_" + tag], dtype=np.float32)
        be = np.asarray(inputs["b_" + tag], dtype=np.float32)
        m = np.asarray(inputs["m_" + tag], dtype=np.float32)
        v = np.asarray(inputs["v_" + tag], dtype=np.float32)
        inv = g / np.sqrt(v + 1e-5)
        Ws[tag] = w * inv[:, None, None, None]
        Bs[tag] = be - m * inv
    init = {
        k: np.asarray(inputs[k], dtype=np.float32).reshape(-1)
        for k in ("h20", "h21", "h30", "h31")
    }

    in_maps = []
    for b in range(B):
        for orient in (0, 1):
            if orient == 0:
                img = xs[b]
                ch = slice(128, 256)
                init_a, init_b = init["h30"], init["h31"]
            else:
                img = xs[b].transpose(0, 2, 1)
                ch = slice(0, 128)
                init_a, init_b = init["h20"], init["h21"]
            xpad = np.pad(img, ((0, 0), (1, 1), (1, 1)))
            wts = np.zeros((128, 15, 128), np.float32)
            consts = np.zeros((128, 8), np.float32)
            for c, tag in enumerate(("z", "h", "s")):
                wc = Ws[tag][ch]  # (128, 64, 3, 3) [cout, cin, ky, kx]
                if orient == 1:
                    wc = wc.transpose(0, 1, 3, 2)
                for dx in range(3):
                    wts[0:64, 3 * c + dx, :] = wc[:, :, 0, dx].T
                    wts[64:128, 3 * c + dx, :] = wc[:, :, 1, dx].T
                wts[0:64, 9 + c, :] = wc[:, :, 2, 0].T
                wts[64:128, 9 + c, :] = wc[:, :, 2, 1].T
                # dy2 dx2 weights in BOTH halves: the K=64 leftover matmuls
                # read it from either row half (paired for concurrency)
                wts[0:64, 12 + c, :] = wc[:, :, 2, 2].T
                wts[64:128, 12 + c, :] = wc[:, :, 2, 2].T
                # biases in (half, chan) layout, separate per scan group
                bg = Bs[tag][ch]
                consts[0:64, c] = bg[0:64]
                consts[64:128, c] = bg[0:64]
                consts[0:64, 3 + c] = bg[64:128]
                consts[64:128, 3 + c] = bg[64:128]
            consts[0:64, 6] = init_a
            consts[64:128, 6] = init_a
            consts[0:64, 7] = init_b
            consts[64:128, 7] = init_b
            cnp = mybir.dt.np(CONV_DT)
            if xpad.dtype != cnp:
                xpad = xpad.astype(cnp)
                wts = wts.astype(cnp)
            # col+1-shifted copy for the row-contiguous x3-upper load
            xpad2 = np.empty_like(xpad)
            xpad2[:, :, :-1] = xpad[:, :, 1:]
            xpad2[:, :, -1] = 0
            in_maps.append(
                {
                    "xp": np.ascontiguousarray(xpad),
                    "xp2": np.ascontiguousarray(xpad2),
                    "wts_a": np.ascontiguousarray(wts[:, :, 0:64]),
                    "wts_b": np.ascontiguousarray(wts[:, :, 64:128]),
                    "consts": consts,
                }
            )
    return in_maps


def _band_list(H):
    R, Rh2 = _R, _R // 2
    bands = [(0, Rh2), (Rh2, Rh2)]
    y = R
    while y < H - R:
        bands.append((y, R))
        y += R
    bands.append((y, Rh2))
    bands.append((y + Rh2, Rh2))
    return bands


def gather_output(core_outs, B, H, W):
    """core_outs: list of 8 arrays (128, H*W) in core order (b-major).

    Device layout: partitions = (half hh in {0,1}, chan c in 0..63);
    free = [band][group g in {0,1}][row r in 0..Rh_b-1][col]. Global row
    of an element = y0_b + hh*Rh_b + r.
    """
    bands = _band_list(H)
    out = np.empty((B, 64, H, W), np.float32)
    for b in range(B):
        for orient in (0, 1):
            flat = core_outs[2 * b + orient].astype(np.float32)
            o = np.empty((64, H, W), np.float32)
            for y0, Rb in bands:
                Rh = Rb // 2
                seg = flat[:, y0 * W : (y0 + Rb) * W]
                seg = seg.reshape(2, 64, 2, Rh, W)  # hh, c, g, r, w
                seg = seg.sum(axis=2)  # sum scan groups: hh, c, r, w
                o[:, y0 : y0 + Rb] = seg.transpose(1, 0, 2, 3).reshape(
                    64, Rb, W
                )
            if orient == 0:
                out[b] = o
            else:
                out[b] += o.transpose(0, 2, 1)
    return out


def kernel(**inputs):
    from concourse.bass_utils import run_bass_kernel_spmd

    _ensure_axon_hooks_importable()
    xs = inputs["xs"]
    B, C, H, W = xs.shape
    # the scan-init fixup ops are only needed for nonzero initial states
    # (the problem spec ships all-zero inits)
    need_fixup = any(
        np.any(np.asarray(inputs[k], dtype=np.float32))
        for k in ("h20", "h21", "h30", "h31")
    )
    nc = _get_nc(H, W, with_init_fixup=need_fixup)
    in_maps = make_in_maps(inputs, H, W)
    res = run_bass_kernel_spmd(nc, in_maps, core_ids=list(range(len(in_maps))))
    outs = [res.results[c]["out"] for c in range(len(in_maps))]
    return gather_output(outs, B, H, W)
